# revision 45
# baseline (speedup 1.0000x reference)
"""ResNet BasicBlock (conv3x3-BN-ReLU-conv3x3-BN-add-ReLU) on 8 TRN2 NeuronCores.

Data-parallel over batch (4 images per core). Convs are implicit GEMM on the
TensorEngine: 9 shifted-window bf16 matmuls accumulated per PSUM row-tile
(inputs/weights bf16, accumulation and stats fp32). Training-mode BatchNorm
is exact sync-BN: per-core (sum, sumsq) partials go through a tiny AllGather,
every core reduces them and applies the affine locally. A throwaway AllGather
at kernel start absorbs the ~50us ncfw collective-init while conv1 runs.
Dummy matmuls on garbage SBUF warm the PE HAM clock gate during the input
DMA, and paced dummy-matmul chains keep it warm across both sync windows
(an idle PE re-throttles to half clock after ~3.4us). EPS*N is folded into
the sumsq stats via a constant accumulator column, so the BN coefficient
chain is 7 ops (ACT Sqrt + DVE reciprocal, no Newton step). conv2 output is
stored bf16. The final relu(scl2*y2 + bia2 + x) mostly runs on the (idle)
TensorEngine — ps = I@x + diag(scl2)@y2 accumulated in PSUM — because
2-tensor-input DVE ops only run at f32 rate; drains are cheap 1-input relus
(ACT Relu-with-bias / DVE add+max), and the output leaves as bf16 in two
half-image DMAs per image (dma_start issue costs ~1.2us queue time each).
The host upcasts to f32.
"""

import functools
from contextlib import ExitStack

import ml_dtypes
import numpy as np

from concourse import bacc, bass, mybir, tile
from concourse.bass_utils import run_bass_kernel_spmd

F32 = mybir.dt.float32
BF16 = mybir.dt.bfloat16
AF = mybir.ActivationFunctionType
ALU = mybir.AluOpType

N_CORES = 8
B, C, H, W = 32, 128, 56, 56
B_SH = B // N_CORES           # 4 images per core
HP, WP = H + 2, W + 2         # 58 (zero-padded)
ROWS = 8                      # output rows per conv tile
TPB = H // ROWS               # 7 tiles per image
NT = B_SH * TPB + 1           # 28 full tiles + 1 (last tile split in half)
N_GLOB = B * H * W            # BN sample count
EPS = 1e-5


def _build():
    nc = bacc.Bacc(
        "TRN2",
        target_bir_lowering=False,
        debug=False,
        enable_asserts=False,
        num_devices=N_CORES,
    )

    xp_d = nc.dram_tensor("xp", [B_SH, C, HP, WP], BF16, kind="ExternalInput")
    id_d = nc.dram_tensor("ident", [C, C], BF16, kind="ExternalInput")
    w1_d = nc.dram_tensor("w1t", [C, 9 * C], BF16, kind="ExternalInput")
    w2_d = nc.dram_tensor("w2t", [C, 9 * C], BF16, kind="ExternalInput")
    g1_d = nc.dram_tensor("g1", [C, 1], F32, kind="ExternalInput")
    b1_d = nc.dram_tensor("b1", [C, 1], F32, kind="ExternalInput")
    g2_d = nc.dram_tensor("g2", [C, 1], F32, kind="ExternalInput")
    b2_d = nc.dram_tensor("b2", [C, 1], F32, kind="ExternalInput")
    out_d = nc.dram_tensor("out", [B_SH, C, H, W], BF16, kind="ExternalOutput")

    with tile.TileContext(nc) as tc, ExitStack() as ctx:
        const = ctx.enter_context(tc.tile_pool(name="const", bufs=1))
        main = ctx.enter_context(tc.tile_pool(name="main", bufs=1))
        scr = ctx.enter_context(tc.tile_pool(name="scr", bufs=1))
        pp = ctx.enter_context(tc.tile_pool(name="pp", bufs=7, space="PSUM"))
        ppw = ctx.enter_context(tc.tile_pool(name="ppw", bufs=1, space="PSUM"))
        dram = ctx.enter_context(tc.tile_pool(name="dram", bufs=1, space="DRAM"))

        # --- collective warm-up -------------------------------------------
        # The first collective pays ~50us of ncfw/driver init. Fire a tiny
        # throwaway AllGather first so that cost overlaps conv1.
        warm_in = dram.tile([8, 32], F32, name="warm_in", tag="warm_in")
        warm_out = dram.tile(
            [N_CORES, 8, 32], F32, name="warm_out", tag="warm_out",
            addr_space="Shared",
        )
        nc.gpsimd.collective_compute(
            "AllGather",
            ALU.bypass,
            ins=[warm_in[:].opt()],
            outs=[warm_out[:].opt()],
            replica_groups=[list(range(N_CORES))],
        )

        # --- PE warm-up: dummy matmuls on garbage SBUF --------------------
        # The HAM clock gate holds the PE at 1.2GHz until ~3.4us of sustained
        # matmul activity. Burn that in on never-written scratch during the
        # input DMA so conv1's first real tiles run at full clock.
        garb_w = scr.tile([C, 128], BF16, name="garb_w", tag="garb_w")
        garb_x = scr.tile([C, 512], BF16, name="garb_x", tag="garb_x")
        nc.vector.memset(garb_w[:], 0.0)
        nc.vector.memset(garb_x[:], 0.0)
        ps_warm = ppw.tile([C, 512], F32, name="ps_warm", tag="ps_warm")
        for _ in range(9):
            nc.tensor.matmul(ps_warm[:], garb_w[:], garb_x[:], start=True, stop=True)

        # --- params + input, in critical-path order -----------------------
        # Chain the big DMAs so conv1's first tiles (w1 + x image 0) land
        # first instead of all transfers sharing bandwidth concurrently.
        from concourse.bass import _add_dep_helper

        xp_sb = []
        prev = None
        for b in range(B_SH):
            t = main.tile([C, HP, WP], BF16, name=f"xp{b}", tag=f"xp{b}")
            if b == 0:
                # split image 0 so conv1's first row-tiles unblock early
                bounds = (0, 12, 34, HP)
                for lo, hi in zip(bounds[:-1], bounds[1:]):
                    d = nc.scalar.dma_start(t[:, lo:hi, :], xp_d[b][:, lo:hi, :])
                    if prev is not None:
                        _add_dep_helper(d.ins, prev.ins, sync=True, reason="dma priority chain")
                    prev = d
            else:
                d = nc.scalar.dma_start(t[:], xp_d[b])
                _add_dep_helper(d.ins, prev.ins, sync=True, reason="dma priority chain")
                prev = d
            xp_sb.append(t)

        w1_sb = const.tile([C, 9 * C], BF16, name="w1_sb", tag="w1_sb")
        nc.sync.dma_start(w1_sb[:], w1_d[:])
        w2_sb = const.tile([C, 9 * C], BF16, name="w2_sb", tag="w2_sb")
        id_sb = const.tile([C, C], BF16, name="id_sb", tag="id_sb")
        bn_par = {}
        for nm in ("g1", "b1", "g2", "b2"):
            bn_par[nm] = const.tile([C, 1], F32, name=f"{nm}_sb", tag=f"{nm}_sb")

        y1p = []  # conv1 raw output, padded buffer (later normalized in place)
        for b in range(B_SH):
            t = main.tile([C, HP, WP], BF16, name=f"y1p{b}", tag=f"y1p{b}")
            # zero the 1-px frame (interior is fully overwritten by conv1)
            nc.gpsimd.memset(t[:, 0, :], 0.0)
            nc.gpsimd.memset(t[:, HP - 1, :], 0.0)
            nc.gpsimd.memset(t[:, :, 0], 0.0)
            nc.gpsimd.memset(t[:, :, WP - 1], 0.0)
            y1p.append(t)

        y2 = []
        for b in range(B_SH):
            t = main.tile([C, H, W], BF16, name=f"y2_{b}", tag=f"y2_{b}")
            y2.append(t)

        # prewarm the ACT sqrt table set (Copy/Relu ride along in every set)
        warm_act = scr.tile([C, 1], F32, name="warm_act", tag="warm_act")
        nc.vector.memset(warm_act[:], 1.0)
        nc.scalar.activation(warm_act[:], warm_act[:], AF.Sqrt)

        # per-tile BN partials, one column per conv tile. The last column of
        # the q tiles is preset to EPS*N/ncores so that after the cross-core
        # reduce, q_glob = sum(y^2) + EPS*N, folding the +EPS into the stats.
        st = {}
        for nm in ("s1", "q1", "s2", "q2"):
            t = scr.tile([C, NT + 1], F32, name=f"st_{nm}", tag=f"st_{nm}")
            nc.vector.memset(
                t[:, NT : NT + 1], EPS * N_GLOB / N_CORES if nm[0] == "q" else 0.0
            )
            st[nm] = t

        sq_scr = scr.tile([C, ROWS, W], BF16, name="sq_scr", tag="sq_scr")

        def conv(x_tiles, w_sb, writer):
            for b in range(B_SH):
                for t in range(TPB):
                    idx = b * TPB + t
                    h0 = t * ROWS
                    last = b == B_SH - 1 and t == TPB - 1
                    # split the final tile so its stats drain is half as long
                    halves = ((0, 4), (4, 8)) if last else ((0, ROWS),)
                    for hi, (r0, r1) in enumerate(halves):
                        nr = r1 - r0
                        ps = pp.tile([C, nr, W], F32, name="ps", tag="ps")
                        for ky in range(3):
                            for kx in range(3):
                                tap = ky * 3 + kx
                                rhs = x_tiles[b][
                                    :, h0 + r0 + ky : h0 + r0 + ky + nr, kx : kx + W
                                ]
                                nc.tensor.matmul(
                                    ps[:],
                                    w_sb[:, tap * C : (tap + 1) * C],
                                    rhs,
                                    start=(tap == 0),
                                    stop=(tap == 8),
                                )
                        writer(b, t, idx + hi, ps, r0, nr)

        def stat_writer(dst_of, s_tile, q_tile):
            def w(b, t, idx, ps, r0, nr):
                # PSUM -> SBUF drain (bf16) + per-channel sum on ScalarE
                dst = dst_of(b, t, r0, nr)
                nc.scalar.activation(
                    dst, ps[:], AF.Copy, accum_out=s_tile[:, idx : idx + 1]
                )
                if idx >= B_SH * TPB - 1:
                    # last tile's halves: sumsq via ACT Square straight from
                    # PSUM, so the whole sync-entry chain (stats -> reduces
                    # -> bounce DMA) stays on the ACT queue with no DVE hop
                    nc.scalar.activation(
                        sq_scr[:, :nr, :], ps[:], AF.Square,
                        accum_out=q_tile[:, idx : idx + 1],
                    )
                else:
                    # sum of squares on VectorE, from the SBUF copy (PSUM has
                    # only one DVE read port; tensor_tensor_reduce faults)
                    nc.vector.scalar_tensor_tensor(
                        sq_scr[:, :nr, :],
                        dst,
                        1.0,
                        dst,
                        ALU.mult,
                        ALU.mult,
                        accum_out=q_tile[:, idx : idx + 1],
                    )

            return w

        def sync_stats(s_tile, q_tile, tag):
            loc = scr.tile([C, 2], F32, name=f"loc{tag}", tag=f"loc{tag}")
            # local reduce as in-place ACT copies with accum_out: they chain
            # on the ACT queue right behind the last conv drain (no DVE->ACT
            # DMA-issuer wake), and the bounce DMA is ACT-issued right after
            nc.scalar.activation(
                s_tile[:], s_tile[:], AF.Copy, accum_out=loc[:, 0:1]
            )
            nc.scalar.activation(
                q_tile[:], q_tile[:], AF.Copy, accum_out=loc[:, 1:2]
            )
            cc_in = dram.tile([C, 2], F32, name=f"ccin{tag}", tag=f"ccin{tag}")
            cc_out = dram.tile(
                [N_CORES, C, 2], F32, name=f"ccout{tag}", tag=f"ccout{tag}",
                addr_space="Shared",
            )
            nc.scalar.dma_start(cc_in[:], loc[:])
            nc.gpsimd.collective_compute(
                "AllGather",
                ALU.bypass,
                ins=[cc_in[:].opt()],
                outs=[cc_out[:].opt()],
                replica_groups=[list(range(N_CORES))],
            )
            graw = scr.tile([C, N_CORES, 2], F32, name=f"graw{tag}", tag=f"graw{tag}")
            nc.sync.dma_start(graw[:], cc_out[:].transpose([1, 0, 2]))
            glob = scr.tile([C, 2], F32, name=f"glob{tag}", tag=f"glob{tag}")
            nc.vector.tensor_reduce(
                glob[:], graw[:].transpose([0, 2, 1]), mybir.AxisListType.X, ALU.add
            )
            return glob, graw

        def warm_keep(start_tile, n_chain, event_tiles):
            """Dummy matmuls that keep the PE HAM clock gate warm across a
            sync window. The first deps on `start_tile` (fires when the sync
            entry work completes); the rest are FIFO-ordered back-to-back;
            then one dummy per event tile bridges the post-collective gaps.
            Event tiles are f32; read them as bf16 bitcast views so the
            dummy matmuls stay all-bf16."""
            for t in [start_tile] + [None] * n_chain + event_tiles:
                rhs = garb_x[:] if t is None else t.bitcast(BF16)
                n = rhs.free_size()
                nc.tensor.matmul(
                    ps_warm[:, :n], garb_w[:], rhs, start=True, stop=True
                )

        def bn_coef(glob, g_sb, b_sb, tag, diag=None):
            # glob = [sum(y), sum(y^2) + EPS*N] per channel
            cf = scr.tile([C, 16], F32, name=f"cf{tag}", tag=f"cf{tag}")
            col = lambda i: cf[:, i : i + 1]
            negm, veps, s0, inv, scl, nscl, bia = (col(i) for i in range(2, 9))
            me = cf[:, 0:2]  # [mean, ex2 + EPS]
            mean, ex2e = cf[:, 0:1], cf[:, 1:2]
            nc.vector.tensor_scalar_mul(me, glob[:], 1.0 / N_GLOB)
            nc.vector.tensor_scalar_mul(negm, mean, -1.0)
            # veps = (ex2 + EPS) - mean^2
            nc.vector.scalar_tensor_tensor(veps, mean, negm, ex2e, ALU.mult, ALU.add)
            # rsqrt(veps) as ACT sqrt + exact-ish DVE reciprocal
            nc.scalar.activation(s0, veps, AF.Sqrt)
            nc.vector.reciprocal(inv, s0)
            nc.vector.tensor_scalar(scl, inv, g_sb[:], None, ALU.mult)
            if diag is not None:
                # build diag(scl) immediately so the PE's final matmuls
                # unblock before the bias half of the chain
                nc.vector.tensor_scalar(diag[:], id_sb[:], scl, None, ALU.mult)
            nc.vector.tensor_scalar_mul(nscl, scl, -1.0)
            # bias = beta - mean * scale
            nc.vector.scalar_tensor_tensor(bia, mean, nscl, b_sb[:], ALU.mult, ALU.add)
            return scl, bia

        # ============ conv1 + BN1 stats ============
        conv(
            xp_sb,
            w1_sb,
            stat_writer(
                lambda b, t, r0, nr: y1p[b][
                    :, 1 + t * ROWS + r0 : 1 + t * ROWS + r0 + nr, 1 : 1 + W
                ],
                st["s1"],
                st["q1"],
            ),
        )
        # deferred: conv2 weights + BN params (not needed until after conv1)
        nc.sync.dma_start(w2_sb[:], w2_d[:])
        nc.sync.dma_start(id_sb[:], id_d[:])
        for nm, dram_t in (("g1", g1_d), ("b1", b1_d), ("g2", g2_d), ("b2", b2_d)):
            nc.sync.dma_start(bn_par[nm][:], dram_t[:])

        glob1, graw1 = sync_stats(st["s1"], st["q1"], "1")
        scl1, bia1 = bn_coef(glob1, bn_par["g1"], bn_par["b1"], "1")
        warm_keep(st["s1"][:, 0:2], 60, [graw1[:, 0, :], glob1[:], scl1])

        # normalize + relu, in place (interior only; border stays zero).
        # image 0 is finely chunked so conv2's first row-tiles unblock asap;
        # later images are normalized just-in-time, interleaved with conv2's
        # drains in ACT queue order.
        norm_chunks = {
            0: [(0, 9), (9, 22), (22, 38), (38, 56)],
            1: [(0, 28), (28, 56)],
            2: [(0, 28), (28, 56)],
            3: [(0, 28), (28, 56)],
        }

        def norm_image(b):
            for ci, (r0, r1) in enumerate(norm_chunks[b]):
                itr = y1p[b][:, 1 + r0 : 1 + r1, 1 : 1 + W]
                if b == 0 and ci == 0:
                    # first chunk on DVE: rides the same queue as the coef
                    # chain (no DVE->ACT wake), so conv2 unblocks sooner
                    nc.vector.tensor_scalar(itr, itr, scl1, bia1, ALU.mult, ALU.add)
                    nc.vector.tensor_scalar(itr, itr, 0.0, None, ALU.max)
                else:
                    nc.scalar.activation(itr, itr, AF.Relu, bias=bia1, scale=scl1)

        # ============ conv2 + BN2 stats ============
        w2writer = stat_writer(
            lambda b, t, r0, nr: y2[b][
                :, t * ROWS + r0 : t * ROWS + r0 + nr, :
            ],
            st["s2"],
            st["q2"],
        )
        for b in range(B_SH):
            norm_image(b)
            for t in range(TPB):
                idx = b * TPB + t
                h0 = t * ROWS
                last = b == B_SH - 1 and t == TPB - 1
                halves = ((0, 4), (4, 8)) if last else ((0, ROWS),)
                for hi, (r0, r1) in enumerate(halves):
                    nr = r1 - r0
                    ps = pp.tile([C, nr, W], F32, name="ps", tag="ps")
                    for ky in range(3):
                        for kx in range(3):
                            tap = ky * 3 + kx
                            rhs = y1p[b][
                                :, h0 + r0 + ky : h0 + r0 + ky + nr, kx : kx + W
                            ]
                            nc.tensor.matmul(
                                ps[:],
                                w2_sb[:, tap * C : (tap + 1) * C],
                                rhs,
                                start=(tap == 0),
                                stop=(tap == 8),
                            )
                    w2writer(b, t, idx + hi, ps, r0, nr)

        glob2, graw2 = sync_stats(st["s2"], st["q2"], "2")
        diag_s = scr.tile([C, C], BF16, name="diag_s", tag="diag_s")
        scl2, bia2 = bn_coef(glob2, bn_par["g2"], bn_par["b2"], "2", diag=diag_s)
        warm_keep(st["s2"][:, 0:2], 68, [graw2[:, 0, :], glob2[:]])

        # ============ final: relu(y2*scl2 + bia2 + x), all bf16 ============
        # The scale and residual add run on the (otherwise idle) TensorEngine:
        # ps = diag(scl2) @ y2_tile, then += I @ x_tile accumulated in PSUM.
        # 2-tensor-input DVE ops only get f32-rate, so this keeps DVE/ACT to
        # cheap 1-input relu drains: DVE (in0+bias2) max 0, or ACT Relu with
        # bias. A DVE stt path takes the last tiles of each image so all
        # three engines finish together; out DMAs chase each drained tile.
        drain_i = 0
        for b in range(B_SH):
            for t in range(TPB):
                r0 = t * ROWS
                ys = y2[b][:, r0 : r0 + ROWS, :]
                xs = xp_sb[b][:, 1 + r0 : 1 + r0 + ROWS, 1 : 1 + W]
                if t < 5 or (b == 3 and t == 5):
                    # TensorEngine path: ps = x + scl2*y2, relu drain. The
                    # identity (residual) matmul goes first — it has no
                    # dependence on the BN coefficients.
                    ps = pp.tile([C, ROWS, W], F32, name="ps", tag="ps")
                    nc.tensor.matmul(ps[:], id_sb[:], xs, start=True, stop=False)
                    nc.tensor.matmul(ps[:], diag_s[:], ys, start=False, stop=True)
                    if drain_i % 3 == 0:
                        nc.vector.tensor_scalar(ys, ps[:], bia2, 0.0, ALU.add, ALU.max)
                    else:
                        nc.scalar.activation(ys, ps[:], AF.Relu, bias=bia2, scale=1.0)
                    drain_i += 1
                else:
                    # DVE path: stt + DVE add+max relu
                    nc.vector.scalar_tensor_tensor(ys, ys, scl2, xs, ALU.mult, ALU.add)
                    nc.vector.tensor_scalar(ys, ys, bia2, 0.0, ALU.add, ALU.max)
            # out-DMA per half-image: dma_start issue costs ~1.2us of queue
            # time each, so few big DMAs beat many small ones
            nc.sync.dma_start(out_d[b][:, 0:28, :], y2[b][:, 0:28, :])
            nc.sync.dma_start(out_d[b][:, 28:56, :], y2[b][:, 28:56, :])

    return nc


@functools.lru_cache(maxsize=1)
def get_nc():
    nc = _build()
    nc.compile()
    return nc


def make_in_maps(x, w1, gamma1, beta1, w2, gamma2, beta2):
    x = np.ascontiguousarray(np.asarray(x, dtype=np.float32))
    xp = np.zeros((B, C, HP, WP), ml_dtypes.bfloat16)
    xp[:, :, 1 : 1 + H, 1 : 1 + W] = x.astype(ml_dtypes.bfloat16)
    # w[o,i,ky,kx] -> [i, (ky,kx,o)] so tap t's lhsT slice is [C_in, C_out]
    w1t = np.ascontiguousarray(
        np.asarray(w1, np.float32).transpose(1, 2, 3, 0)
    ).reshape(C, 9 * C).astype(ml_dtypes.bfloat16)
    w2t = np.ascontiguousarray(
        np.asarray(w2, np.float32).transpose(1, 2, 3, 0)
    ).reshape(C, 9 * C).astype(ml_dtypes.bfloat16)
    ident = np.ascontiguousarray(np.eye(C, dtype=ml_dtypes.bfloat16))
    g1 = np.ascontiguousarray(np.asarray(gamma1, np.float32).reshape(C, 1))
    b1 = np.ascontiguousarray(np.asarray(beta1, np.float32).reshape(C, 1))
    g2 = np.ascontiguousarray(np.asarray(gamma2, np.float32).reshape(C, 1))
    b2 = np.ascontiguousarray(np.asarray(beta2, np.float32).reshape(C, 1))
    maps = []
    for i in range(N_CORES):
        maps.append(
            {
                "xp": np.ascontiguousarray(xp[i * B_SH : (i + 1) * B_SH]),
                "ident": ident,
                "w1t": w1t,
                "w2t": w2t,
                "g1": g1,
                "b1": b1,
                "g2": g2,
                "b2": b2,
            }
        )
    return maps


def run(in_maps, trace=False, **kwargs):
    nc = get_nc()
    return run_bass_kernel_spmd(
        nc, in_maps, core_ids=list(range(N_CORES)), trace=trace, **kwargs
    )


def kernel(x, w1, gamma1, beta1, w2, gamma2, beta2):
    maps = make_in_maps(x, w1, gamma1, beta1, w2, gamma2, beta2)
    res = run(maps)
    out = np.concatenate([res.results[i]["out"] for i in range(N_CORES)], axis=0)
    return np.ascontiguousarray(np.asarray(out, dtype=np.float32))


# revision 46
# speedup vs baseline: 1.0228x; 1.0228x over previous
"""ResNet BasicBlock (conv3x3-BN-ReLU-conv3x3-BN-add-ReLU) on 8 TRN2 NeuronCores.

Data-parallel over batch (4 images per core). Convs are implicit GEMM on the
TensorEngine: 9 shifted-window bf16 matmuls accumulated per PSUM row-tile
(inputs/weights bf16, accumulation and stats fp32). Training-mode BatchNorm
is exact sync-BN: per-core (sum, sumsq) partials go through a tiny AllGather,
every core reduces them and applies the affine locally. A throwaway AllGather
at kernel start absorbs the ~50us ncfw collective-init while conv1 runs.
Dummy matmuls on garbage SBUF warm the PE HAM clock gate during the input
DMA, and paced dummy-matmul chains keep it warm across both sync windows
(an idle PE re-throttles to half clock after ~3.4us). EPS*N is folded into
the sumsq stats via a constant accumulator column, so the BN coefficient
chain is 7 ops (ACT Sqrt + DVE reciprocal, no Newton step). conv2 output is
stored bf16. The final relu(scl2*y2 + bia2 + x) mostly runs on the (idle)
TensorEngine — ps = I@x + diag(scl2)@y2 accumulated in PSUM — because
2-tensor-input DVE ops only run at f32 rate; drains are cheap 1-input relus
(ACT Relu-with-bias / DVE add+max), and the output leaves as bf16 in two
half-image DMAs per image (dma_start issue costs ~1.2us queue time each).
The host upcasts to f32.
"""

import functools
from contextlib import ExitStack

import ml_dtypes
import numpy as np

from concourse import bacc, bass, mybir, tile
from concourse.bass_utils import run_bass_kernel_spmd

F32 = mybir.dt.float32
BF16 = mybir.dt.bfloat16
AF = mybir.ActivationFunctionType
ALU = mybir.AluOpType

N_CORES = 8
B, C, H, W = 32, 128, 56, 56
B_SH = B // N_CORES           # 4 images per core
HP, WP = H + 2, W + 2         # 58 (zero-padded)
ROWS = 8                      # output rows per conv tile
TPB = H // ROWS               # 7 tiles per image
NT = B_SH * TPB + 1           # 28 full tiles + 1 (last tile split in half)
N_GLOB = B * H * W            # BN sample count
EPS = 1e-5


def _build():
    nc = bacc.Bacc(
        "TRN2",
        target_bir_lowering=False,
        debug=False,
        enable_asserts=False,
        num_devices=N_CORES,
    )

    xp_d = nc.dram_tensor("xp", [B_SH, C, HP, WP], BF16, kind="ExternalInput")
    id_d = nc.dram_tensor("ident", [C, C], BF16, kind="ExternalInput")
    w1_d = nc.dram_tensor("w1t", [C, 9 * C], BF16, kind="ExternalInput")
    w2_d = nc.dram_tensor("w2t", [C, 9 * C], BF16, kind="ExternalInput")
    g1_d = nc.dram_tensor("g1", [C, 1], F32, kind="ExternalInput")
    b1_d = nc.dram_tensor("b1", [C, 1], F32, kind="ExternalInput")
    g2_d = nc.dram_tensor("g2", [C, 1], F32, kind="ExternalInput")
    b2_d = nc.dram_tensor("b2", [C, 1], F32, kind="ExternalInput")
    out_d = nc.dram_tensor("out", [B_SH, C, H, W], BF16, kind="ExternalOutput")

    with tile.TileContext(nc) as tc, ExitStack() as ctx:
        const = ctx.enter_context(tc.tile_pool(name="const", bufs=1))
        main = ctx.enter_context(tc.tile_pool(name="main", bufs=1))
        scr = ctx.enter_context(tc.tile_pool(name="scr", bufs=1))
        pp = ctx.enter_context(tc.tile_pool(name="pp", bufs=7, space="PSUM"))
        ppw = ctx.enter_context(tc.tile_pool(name="ppw", bufs=1, space="PSUM"))
        dram = ctx.enter_context(tc.tile_pool(name="dram", bufs=1, space="DRAM"))

        # --- collective warm-up -------------------------------------------
        # The first collective pays ~50us of ncfw/driver init. Fire a tiny
        # throwaway AllGather first so that cost overlaps conv1.
        warm_in = dram.tile([8, 32], F32, name="warm_in", tag="warm_in")
        warm_out = dram.tile(
            [N_CORES, 8, 32], F32, name="warm_out", tag="warm_out",
            addr_space="Shared",
        )
        nc.gpsimd.collective_compute(
            "AllGather",
            ALU.bypass,
            ins=[warm_in[:].opt()],
            outs=[warm_out[:].opt()],
            replica_groups=[list(range(N_CORES))],
        )

        # --- PE warm-up: dummy matmuls on garbage SBUF --------------------
        # The HAM clock gate holds the PE at 1.2GHz until ~3.4us of sustained
        # matmul activity. Burn that in on never-written scratch during the
        # input DMA so conv1's first real tiles run at full clock.
        garb_w = scr.tile([C, 128], BF16, name="garb_w", tag="garb_w")
        garb_x = scr.tile([C, 512], BF16, name="garb_x", tag="garb_x")
        nc.vector.memset(garb_w[:], 0.0)
        nc.vector.memset(garb_x[:], 0.0)
        ps_warm = ppw.tile([C, 512], F32, name="ps_warm", tag="ps_warm")
        for _ in range(9):
            nc.tensor.matmul(ps_warm[:], garb_w[:], garb_x[:], start=True, stop=True)

        # --- params + input, in critical-path order -----------------------
        # Chain the big DMAs so conv1's first tiles (w1 + x image 0) land
        # first instead of all transfers sharing bandwidth concurrently.
        from concourse.bass import _add_dep_helper

        xp_sb = []
        prev = None
        for b in range(B_SH):
            t = main.tile([C, HP, WP], BF16, name=f"xp{b}", tag=f"xp{b}")
            if b == 0:
                # split image 0 so conv1's first row-tiles unblock early
                bounds = (0, 12, 34, HP)
                for lo, hi in zip(bounds[:-1], bounds[1:]):
                    d = nc.scalar.dma_start(t[:, lo:hi, :], xp_d[b][:, lo:hi, :])
                    if prev is not None:
                        _add_dep_helper(d.ins, prev.ins, sync=True, reason="dma priority chain")
                    prev = d
            else:
                d = nc.scalar.dma_start(t[:], xp_d[b])
                _add_dep_helper(d.ins, prev.ins, sync=True, reason="dma priority chain")
                prev = d
            xp_sb.append(t)

        w1_sb = const.tile([C, 9 * C], BF16, name="w1_sb", tag="w1_sb")
        nc.sync.dma_start(w1_sb[:], w1_d[:])
        w2_sb = const.tile([C, 9 * C], BF16, name="w2_sb", tag="w2_sb")
        id_sb = const.tile([C, C], BF16, name="id_sb", tag="id_sb")
        bn_par = {}
        for nm in ("g1", "b1", "g2", "b2"):
            bn_par[nm] = const.tile([C, 1], F32, name=f"{nm}_sb", tag=f"{nm}_sb")

        y1p = []  # conv1 raw output, padded buffer (later normalized in place)
        for b in range(B_SH):
            t = main.tile([C, HP, WP], BF16, name=f"y1p{b}", tag=f"y1p{b}")
            # zero the 1-px frame (interior is fully overwritten by conv1)
            nc.gpsimd.memset(t[:, 0, :], 0.0)
            nc.gpsimd.memset(t[:, HP - 1, :], 0.0)
            nc.gpsimd.memset(t[:, :, 0], 0.0)
            nc.gpsimd.memset(t[:, :, WP - 1], 0.0)
            y1p.append(t)

        y2 = []
        for b in range(B_SH):
            t = main.tile([C, H, W], BF16, name=f"y2_{b}", tag=f"y2_{b}")
            y2.append(t)

        # prewarm the ACT sqrt table set (Copy/Relu ride along in every set)
        warm_act = scr.tile([C, 1], F32, name="warm_act", tag="warm_act")
        nc.vector.memset(warm_act[:], 1.0)
        nc.scalar.activation(warm_act[:], warm_act[:], AF.Sqrt)

        # per-tile BN partials, one column per conv tile. The last column of
        # the q tiles is preset to EPS*N/ncores so that after the cross-core
        # reduce, q_glob = sum(y^2) + EPS*N, folding the +EPS into the stats.
        st = {}
        for nm in ("s1", "q1", "s2", "q2"):
            t = scr.tile([C, NT + 1], F32, name=f"st_{nm}", tag=f"st_{nm}")
            nc.vector.memset(
                t[:, NT : NT + 1], EPS * N_GLOB / N_CORES if nm[0] == "q" else 0.0
            )
            st[nm] = t

        sq_scr = scr.tile([C, ROWS, W], BF16, name="sq_scr", tag="sq_scr")

        def conv(x_tiles, w_sb, writer):
            for b in range(B_SH):
                for t in range(TPB):
                    idx = b * TPB + t
                    h0 = t * ROWS
                    last = b == B_SH - 1 and t == TPB - 1
                    # split the final tile so its stats drain is half as long
                    halves = ((0, 4), (4, 8)) if last else ((0, ROWS),)
                    for hi, (r0, r1) in enumerate(halves):
                        nr = r1 - r0
                        ps = pp.tile([C, nr, W], F32, name="ps", tag="ps")
                        for ky in range(3):
                            for kx in range(3):
                                tap = ky * 3 + kx
                                rhs = x_tiles[b][
                                    :, h0 + r0 + ky : h0 + r0 + ky + nr, kx : kx + W
                                ]
                                nc.tensor.matmul(
                                    ps[:],
                                    w_sb[:, tap * C : (tap + 1) * C],
                                    rhs,
                                    start=(tap == 0),
                                    stop=(tap == 8),
                                )
                        writer(b, t, idx + hi, ps, r0, nr)

        def stat_writer(dst_of, s_tile, q_tile):
            def w(b, t, idx, ps, r0, nr):
                # PSUM -> SBUF drain (bf16) + per-channel sum on ScalarE
                dst = dst_of(b, t, r0, nr)
                nc.scalar.activation(
                    dst, ps[:], AF.Copy, accum_out=s_tile[:, idx : idx + 1]
                )
                if idx >= B_SH * TPB - 1:
                    # last tile's halves: sumsq via ACT Square straight from
                    # PSUM, so the whole sync-entry chain (stats -> reduces
                    # -> bounce DMA) stays on the ACT queue with no DVE hop
                    nc.scalar.activation(
                        sq_scr[:, :nr, :], ps[:], AF.Square,
                        accum_out=q_tile[:, idx : idx + 1],
                    )
                else:
                    # sum of squares on VectorE, from the SBUF copy (PSUM has
                    # only one DVE read port; tensor_tensor_reduce faults)
                    nc.vector.scalar_tensor_tensor(
                        sq_scr[:, :nr, :],
                        dst,
                        1.0,
                        dst,
                        ALU.mult,
                        ALU.mult,
                        accum_out=q_tile[:, idx : idx + 1],
                    )

            return w

        def sync_stats(s_tile, q_tile, tag):
            loc = scr.tile([C, 2], F32, name=f"loc{tag}", tag=f"loc{tag}")
            # local reduce as in-place ACT copies with accum_out: they chain
            # on the ACT queue right behind the last conv drain (no DVE->ACT
            # DMA-issuer wake), and the bounce DMA is ACT-issued right after
            nc.scalar.activation(
                s_tile[:], s_tile[:], AF.Copy, accum_out=loc[:, 0:1]
            )
            nc.scalar.activation(
                q_tile[:], q_tile[:], AF.Copy, accum_out=loc[:, 1:2]
            )
            cc_in = dram.tile([C, 2], F32, name=f"ccin{tag}", tag=f"ccin{tag}")
            cc_out = dram.tile(
                [N_CORES, C, 2], F32, name=f"ccout{tag}", tag=f"ccout{tag}",
                addr_space="Shared",
            )
            nc.scalar.dma_start(cc_in[:], loc[:])
            nc.gpsimd.collective_compute(
                "AllGather",
                ALU.bypass,
                ins=[cc_in[:].opt()],
                outs=[cc_out[:].opt()],
                replica_groups=[list(range(N_CORES))],
            )
            graw = scr.tile([C, N_CORES, 2], F32, name=f"graw{tag}", tag=f"graw{tag}")
            nc.sync.dma_start(graw[:], cc_out[:].transpose([1, 0, 2]))
            glob = scr.tile([C, 2], F32, name=f"glob{tag}", tag=f"glob{tag}")
            nc.vector.tensor_reduce(
                glob[:], graw[:].transpose([0, 2, 1]), mybir.AxisListType.X, ALU.add
            )
            return glob, graw

        def warm_keep(start_tile, n_chain, event_tiles):
            """Dummy matmuls that keep the PE HAM clock gate warm across a
            sync window. The first deps on `start_tile` (fires when the sync
            entry work completes); the rest are FIFO-ordered back-to-back;
            then one dummy per event tile bridges the post-collective gaps.
            Event tiles are f32; read them as bf16 bitcast views so the
            dummy matmuls stay all-bf16."""
            for t in [start_tile] + [None] * n_chain + event_tiles:
                rhs = garb_x[:] if t is None else t.bitcast(BF16)
                n = rhs.free_size()
                nc.tensor.matmul(
                    ps_warm[:, :n], garb_w[:], rhs, start=True, stop=True
                )

        def bn_coef(glob, g_sb, b_sb, tag):
            # glob = [sum(y), sum(y^2) + EPS*N] per channel
            cf = scr.tile([C, 16], F32, name=f"cf{tag}", tag=f"cf{tag}")
            col = lambda i: cf[:, i : i + 1]
            negm, veps, s0, inv, scl, nscl, bia = (col(i) for i in range(2, 9))
            me = cf[:, 0:2]  # [mean, ex2 + EPS]
            mean, ex2e = cf[:, 0:1], cf[:, 1:2]
            nc.vector.tensor_scalar_mul(me, glob[:], 1.0 / N_GLOB)
            nc.vector.tensor_scalar_mul(negm, mean, -1.0)
            # veps = (ex2 + EPS) - mean^2
            nc.vector.scalar_tensor_tensor(veps, mean, negm, ex2e, ALU.mult, ALU.add)
            # rsqrt(veps) as ACT sqrt + exact-ish DVE reciprocal
            nc.scalar.activation(s0, veps, AF.Sqrt)
            nc.vector.reciprocal(inv, s0)
            nc.vector.tensor_scalar(scl, inv, g_sb[:], None, ALU.mult)
            nc.vector.tensor_scalar_mul(nscl, scl, -1.0)
            # bias = beta - mean * scale
            nc.vector.scalar_tensor_tensor(bia, mean, nscl, b_sb[:], ALU.mult, ALU.add)
            return scl, bia

        # ============ conv1 + BN1 stats ============
        conv(
            xp_sb,
            w1_sb,
            stat_writer(
                lambda b, t, r0, nr: y1p[b][
                    :, 1 + t * ROWS + r0 : 1 + t * ROWS + r0 + nr, 1 : 1 + W
                ],
                st["s1"],
                st["q1"],
            ),
        )
        # deferred: conv2 weights + BN params (not needed until after conv1)
        nc.sync.dma_start(w2_sb[:], w2_d[:])
        nc.sync.dma_start(id_sb[:], id_d[:])
        for nm, dram_t in (("g1", g1_d), ("b1", b1_d), ("g2", g2_d), ("b2", b2_d)):
            nc.sync.dma_start(bn_par[nm][:], dram_t[:])

        glob1, graw1 = sync_stats(st["s1"], st["q1"], "1")
        scl1, bia1 = bn_coef(glob1, bn_par["g1"], bn_par["b1"], "1")
        warm_keep(st["s1"][:, 0:2], 60, [graw1[:, 0, :], glob1[:], scl1])

        # normalize + relu, in place (interior only; border stays zero).
        # image 0 is finely chunked so conv2's first row-tiles unblock asap;
        # later images are normalized just-in-time, interleaved with conv2's
        # drains in ACT queue order.
        norm_chunks = {
            0: [(0, 9), (9, 22), (22, 38), (38, 56)],
            1: [(0, 28), (28, 56)],
            2: [(0, 28), (28, 56)],
            3: [(0, 28), (28, 56)],
        }

        def norm_image(b):
            for r0, r1 in norm_chunks[b]:
                itr = y1p[b][:, 1 + r0 : 1 + r1, 1 : 1 + W]
                nc.scalar.activation(itr, itr, AF.Relu, bias=bia1, scale=scl1)

        # ============ conv2 + BN2 stats ============
        w2writer = stat_writer(
            lambda b, t, r0, nr: y2[b][
                :, t * ROWS + r0 : t * ROWS + r0 + nr, :
            ],
            st["s2"],
            st["q2"],
        )
        for b in range(B_SH):
            norm_image(b)
            for t in range(TPB):
                idx = b * TPB + t
                h0 = t * ROWS
                last = b == B_SH - 1 and t == TPB - 1
                halves = ((0, 4), (4, 8)) if last else ((0, ROWS),)
                for hi, (r0, r1) in enumerate(halves):
                    nr = r1 - r0
                    ps = pp.tile([C, nr, W], F32, name="ps", tag="ps")
                    for ky in range(3):
                        for kx in range(3):
                            tap = ky * 3 + kx
                            rhs = y1p[b][
                                :, h0 + r0 + ky : h0 + r0 + ky + nr, kx : kx + W
                            ]
                            nc.tensor.matmul(
                                ps[:],
                                w2_sb[:, tap * C : (tap + 1) * C],
                                rhs,
                                start=(tap == 0),
                                stop=(tap == 8),
                            )
                    w2writer(b, t, idx + hi, ps, r0, nr)

        glob2, graw2 = sync_stats(st["s2"], st["q2"], "2")
        scl2, bia2 = bn_coef(glob2, bn_par["g2"], bn_par["b2"], "2")
        warm_keep(st["s2"][:, 0:2], 68, [graw2[:, 0, :], glob2[:]])

        # ============ final: relu(y2*scl2 + bia2 + x), all bf16 ============
        # The scale and residual add run on the (otherwise idle) TensorEngine:
        # ps = diag(scl2) @ y2_tile, then += I @ x_tile accumulated in PSUM.
        # 2-tensor-input DVE ops only get f32-rate, so this keeps DVE/ACT to
        # cheap 1-input relu drains: DVE (in0+bias2) max 0, or ACT Relu with
        # bias. A DVE stt path takes the last tiles of each image so all
        # three engines finish together; out DMAs chase each drained tile.
        diag_s = scr.tile([C, C], BF16, name="diag_s", tag="diag_s")
        nc.vector.tensor_scalar(diag_s[:], id_sb[:], scl2, None, ALU.mult)
        drain_i = 0
        for b in range(B_SH):
            for t in range(TPB):
                r0 = t * ROWS
                ys = y2[b][:, r0 : r0 + ROWS, :]
                xs = xp_sb[b][:, 1 + r0 : 1 + r0 + ROWS, 1 : 1 + W]
                if t < 5:
                    # TensorEngine path: ps = x + scl2*y2, relu drain. The
                    # identity (residual) matmul goes first — it has no
                    # dependence on the BN coefficients.
                    ps = pp.tile([C, ROWS, W], F32, name="ps", tag="ps")
                    nc.tensor.matmul(ps[:], id_sb[:], xs, start=True, stop=False)
                    nc.tensor.matmul(ps[:], diag_s[:], ys, start=False, stop=True)
                    if drain_i % 4 == 0:
                        nc.vector.tensor_scalar(ys, ps[:], bia2, 0.0, ALU.add, ALU.max)
                    else:
                        nc.scalar.activation(ys, ps[:], AF.Relu, bias=bia2, scale=1.0)
                    drain_i += 1
                else:
                    # DVE path: stt + DVE add+max relu
                    nc.vector.scalar_tensor_tensor(ys, ys, scl2, xs, ALU.mult, ALU.add)
                    nc.vector.tensor_scalar(ys, ys, bia2, 0.0, ALU.add, ALU.max)
            # out-DMA per half-image: dma_start issue costs ~1.2us of queue
            # time each, so few big DMAs beat many small ones
            nc.sync.dma_start(out_d[b][:, 0:28, :], y2[b][:, 0:28, :])
            nc.sync.dma_start(out_d[b][:, 28:56, :], y2[b][:, 28:56, :])

    return nc


@functools.lru_cache(maxsize=1)
def get_nc():
    nc = _build()
    nc.compile()
    return nc


def make_in_maps(x, w1, gamma1, beta1, w2, gamma2, beta2):
    x = np.ascontiguousarray(np.asarray(x, dtype=np.float32))
    xp = np.zeros((B, C, HP, WP), ml_dtypes.bfloat16)
    xp[:, :, 1 : 1 + H, 1 : 1 + W] = x.astype(ml_dtypes.bfloat16)
    # w[o,i,ky,kx] -> [i, (ky,kx,o)] so tap t's lhsT slice is [C_in, C_out]
    w1t = np.ascontiguousarray(
        np.asarray(w1, np.float32).transpose(1, 2, 3, 0)
    ).reshape(C, 9 * C).astype(ml_dtypes.bfloat16)
    w2t = np.ascontiguousarray(
        np.asarray(w2, np.float32).transpose(1, 2, 3, 0)
    ).reshape(C, 9 * C).astype(ml_dtypes.bfloat16)
    ident = np.ascontiguousarray(np.eye(C, dtype=ml_dtypes.bfloat16))
    g1 = np.ascontiguousarray(np.asarray(gamma1, np.float32).reshape(C, 1))
    b1 = np.ascontiguousarray(np.asarray(beta1, np.float32).reshape(C, 1))
    g2 = np.ascontiguousarray(np.asarray(gamma2, np.float32).reshape(C, 1))
    b2 = np.ascontiguousarray(np.asarray(beta2, np.float32).reshape(C, 1))
    maps = []
    for i in range(N_CORES):
        maps.append(
            {
                "xp": np.ascontiguousarray(xp[i * B_SH : (i + 1) * B_SH]),
                "ident": ident,
                "w1t": w1t,
                "w2t": w2t,
                "g1": g1,
                "b1": b1,
                "g2": g2,
                "b2": b2,
            }
        )
    return maps


def run(in_maps, trace=False, **kwargs):
    nc = get_nc()
    return run_bass_kernel_spmd(
        nc, in_maps, core_ids=list(range(N_CORES)), trace=trace, **kwargs
    )


def kernel(x, w1, gamma1, beta1, w2, gamma2, beta2):
    maps = make_in_maps(x, w1, gamma1, beta1, w2, gamma2, beta2)
    res = run(maps)
    out = np.concatenate([res.results[i]["out"] for i in range(N_CORES)], axis=0)
    return np.ascontiguousarray(np.asarray(out, dtype=np.float32))
